# revision 7
# baseline (speedup 1.0000x reference)
"""GPTQ int4 dequant + matmul + bias + residual for Trainium2, 8 NeuronCores.

Problem (hardcoded): input [4,2048,4096] f32, qweight int32 [512,4096] (8 int4
along K per int32), scales [32,4096], qzeros int32 [32,512] (8 int4 along N),
g_idx = arange(4096)//128 (contiguous groups), bias [4096], residual
[4,2048,4096].  out = x @ dequant(W) + bias + residual.

Sharding: data-parallel over tokens (M = B*S = 8192 rows -> 1024 rows/core);
every core keeps the full weight.

v2 strategy: the device does ONLY roofline work.  All reshuffling moves to
host numpy prep:
  - x is cast to f16 and pre-transposed/permuted on host into
    xt[kp, mi, t, j, m] = x[mi*128+m, 1024t+8kp+j], so PE lhsT tiles load
    straight from DMA (no on-device transposes, casts, or copies).
  - The GPTQ zero-point correction and bias fold into the residual on host:
    resid' = residual + bias - xg @ ((qz+1)*scales), xg = per-group sums of x.
  - Scales are pre-broadcast on host to scb[t, p, n] = scales[8t+p//16, n]
    (f16), removing the on-device E16 indicator matmuls.
Device steady state per 512-col chunk: 4 packed-weight DMAs, 32 DVE
shift+and passes, 32 DVE scale-mult passes -> wdq f16 tiles; then 8 PSUM
groups (one per 128-row M-tile) accumulate 32 matmuls each, interleaved
mi-innermost so a single dequanted weight tile feeds 8 back-to-back MMs;
epilogue: ACT copies PSUM->SBUF (frees the bank fast), DVE adds residual
in place, DMA out.
"""

import numpy as np

import concourse.bass as bass
import concourse.mybir as mybir
import concourse.tile as tile
from concourse import bacc
from concourse.alu_op_type import AluOpType
from concourse.bass_utils import run_bass_kernel_spmd

F32 = mybir.dt.float32
F16 = mybir.dt.float16
I32 = mybir.dt.int32

B, S, K, N = 4, 2048, 4096, 4096
PACK = 8
GROUP = 128
G = K // GROUP          # 32 groups
NCORES = 8
M = (B * S) // NCORES   # 1024 rows per core
CHUNK = 512
NCH = N // CHUNK        # 8
TS = K // 1024          # 4 super-tiles of 1024 k
MT = M // 128           # 8 M-tiles per core


def _build():
    nc = bacc.Bacc(name="gptq_mm2", dynamic_dma_scratch_size=2048)
    xt_d = nc.declare_dram_parameter("xt", [128, MT, TS, PACK, 128], F16, isOutput=False)
    wq_d = nc.declare_dram_parameter("wq", [K // PACK, N], I32, isOutput=False)
    scb_d = nc.declare_dram_parameter("scb", [TS, 128, N], F16, isOutput=False)
    res_d = nc.declare_dram_parameter("resid", [M, N], F32, isOutput=False)
    out_d = nc.declare_dram_parameter("out", [M, N], F32, isOutput=True)

    with tile.TileContext(nc) as tc:
        with (
            tc.tile_pool(name="const", bufs=1) as const,
            tc.tile_pool(name="pk", bufs=6) as pkp,
            tc.tile_pool(name="u", bufs=4) as up,
            tc.tile_pool(name="wdq", bufs=40) as wdqp,
            tc.tile_pool(name="rt", bufs=9) as rtp,
            tc.tile_pool(name="ob", bufs=4) as obp,
            tc.tile_pool(name="ps", bufs=8, space="PSUM") as psp,
        ):
            scb_sb = const.tile([128, TS, N], F16, tag="scb")
            xt_sb = const.tile([128, MT, TS, PACK, 128], F16, tag="xt")
            # Pre-warm the PE clock (HAM un-throttles after ~3.4us of
            # activity) with junk matmuls while the first DMAs land.
            junk = const.tile([128, CHUNK], F16, tag="junk")
            nc.vector.memset(junk[:], 0.0)
            jps = psp.tile([128, CHUNK], F32, tag="ps")
            for _ in range(16):
                nc.tensor.matmul(
                    jps[:], lhsT=junk[:, 0:128], rhs=junk[:],
                    start=True, stop=True,
                )
            # x streams on the ACT HWDGE queue so it never delays the
            # chunk-0 weight/scale DMAs on the sync queue.
            for mi in range(MT):
                nc.scalar.dma_start(out=xt_sb[:, mi], in_=xt_d[:, mi])

            def epilogue(c, mi, ps, rt):
                cs = slice(c * CHUNK, (c + 1) * CHUNK)
                ob = obp.tile([128, CHUNK], F32, tag="ob")
                nc.scalar.copy(ob[:], ps[:])
                nc.vector.tensor_tensor(
                    out=ob[:], in0=ob[:], in1=rt[:], op=AluOpType.add,
                )
                nc.scalar.dma_start(
                    out=out_d[mi * 128:(mi + 1) * 128, cs], in_=ob[:]
                )

            for c in range(NCH):
                cs = slice(c * CHUNK, (c + 1) * CHUNK)
                pks = []
                for t in range(TS):
                    pk = pkp.tile([128, CHUNK], I32, tag="pk")
                    nc.sync.dma_start(out=pk[:], in_=wq_d[128 * t:128 * (t + 1), cs])
                    pks.append(pk)
                    nc.sync.dma_start(
                        out=scb_sb[:, t, cs], in_=scb_d[t][:, cs]
                    )
                rts = []
                for mi in range(MT):
                    rt = rtp.tile([128, CHUNK], F32, tag="rt")
                    nc.scalar.dma_start(
                        out=rt[:], in_=res_d[mi * 128:(mi + 1) * 128, cs]
                    )
                    rts.append(rt)
                wdqs = []
                for t in range(TS):
                    for j in range(PACK):
                        u = up.tile([128, CHUNK], I32, tag="u")
                        nc.vector.tensor_scalar(
                            out=u[:], in0=pks[t][:],
                            scalar1=4 * j, scalar2=0xF,
                            op0=AluOpType.logical_shift_right,
                            op1=AluOpType.bitwise_and,
                        )
                        wdq = wdqp.tile([128, CHUNK], F16, tag="wdq")
                        nc.vector.tensor_tensor(
                            out=wdq[:], in0=u[:], in1=scb_sb[:, t, cs],
                            op=AluOpType.mult,
                        )
                        wdqs.append(wdq)
                pss = []
                for mi in range(MT):
                    ps = psp.tile([128, CHUNK], F32, tag="ps")
                    pss.append(ps)
                if c < NCH - 1:
                    # mi-innermost: each dequanted weight tile feeds 8
                    # back-to-back MMs, so chunk 0 ramps at DMA/DVE pace.
                    for idx in range(TS * PACK):
                        t, j = divmod(idx, PACK)
                        for mi in range(MT):
                            nc.tensor.matmul(
                                pss[mi][:],
                                lhsT=xt_sb[:, mi, t, j, :],
                                rhs=wdqs[idx][:],
                                start=(idx == 0), stop=(idx == TS * PACK - 1),
                            )
                    for mi in range(MT):
                        epilogue(c, mi, pss[mi], rts[mi])
                else:
                    # last chunk: mi-outer so PSUM groups finish staggered
                    # and only one epilogue lands after the final MM.
                    for mi in range(MT):
                        for idx in range(TS * PACK):
                            t, j = divmod(idx, PACK)
                            nc.tensor.matmul(
                                pss[mi][:],
                                lhsT=xt_sb[:, mi, t, j, :],
                                rhs=wdqs[idx][:],
                                start=(idx == 0), stop=(idx == TS * PACK - 1),
                            )
                        epilogue(c, mi, pss[mi], rts[mi])

    nc.finalize()
    return nc


_NC_CACHE = None


def _get_nc():
    global _NC_CACHE
    if _NC_CACHE is None:
        _NC_CACHE = _build()
    return _NC_CACHE


def _host_prep(input, weight, weight_scales, weight_zeros, bias, residual):
    """All host-side reshuffling; returns per-core in_maps."""
    x = np.ascontiguousarray(input.reshape(B * S, K))
    x16 = x.astype(np.float16)

    # zero-point + bias correction folded into residual:
    #   out = x @ (qw*s) - xg @ ((qz+1)*s) + bias + residual
    jj = (np.arange(PACK, dtype=np.int32) * 4)
    qz = ((weight_zeros[:, :, None] >> jj[None, None, :]) & 0xF).reshape(G, N)
    nzs = (qz + 1).astype(np.float32) * weight_scales          # [G, N]
    xg = x.reshape(B * S, G, GROUP).sum(axis=2, dtype=np.float32)  # [M_all, G]
    resid2 = (
        residual.reshape(B * S, N) + bias[None, :] - xg @ nzs
    ).astype(np.float32)

    # scales broadcast: scb[t, p, n] = scales[8t + p//16, n], f16
    sc16 = weight_scales.astype(np.float16)
    scb = np.ascontiguousarray(
        np.repeat(sc16.reshape(TS, PACK, 1, N), 16, axis=2).reshape(TS, 128, N)
    )

    in_maps = []
    for ci in range(NCORES):
        rs = slice(ci * M, (ci + 1) * M)
        xc = x16[rs].reshape(MT, 128, TS, 128, PACK)   # [mi, m, t, kp, j]
        xt = np.ascontiguousarray(xc.transpose(3, 0, 2, 4, 1))  # [kp,mi,t,j,m]
        in_maps.append(dict(
            xt=xt,
            wq=np.ascontiguousarray(weight),
            scb=scb,
            resid=np.ascontiguousarray(resid2[rs]),
        ))
    return in_maps


def kernel(input, weight, weight_scales, weight_zeros, g_idx, bias, residual):
    input = np.asarray(input, dtype=np.float32)
    weight = np.ascontiguousarray(np.asarray(weight, dtype=np.int32))
    weight_scales = np.ascontiguousarray(np.asarray(weight_scales, dtype=np.float32))
    weight_zeros = np.asarray(weight_zeros, dtype=np.int32)
    g_idx = np.asarray(g_idx, dtype=np.int32)
    bias = np.asarray(bias, dtype=np.float32)
    residual = np.asarray(residual, dtype=np.float32)

    assert input.shape == (B, S, K) and weight.shape == (K // PACK, N)
    assert np.array_equal(g_idx, np.arange(K, dtype=np.int32) // GROUP), \
        "kernel assumes contiguous GPTQ groups (g_idx == arange(K)//group_size)"

    in_maps = _host_prep(input, weight, weight_scales, weight_zeros, bias, residual)
    nc = _get_nc()
    res = run_bass_kernel_spmd(nc, in_maps, core_ids=list(range(NCORES)))
    out = np.concatenate([r["out"] for r in res.results], axis=0)
    return out.reshape(B, S, N)


# revision 8
# speedup vs baseline: 1.0197x; 1.0197x over previous
"""GPTQ int4 dequant + matmul + bias + residual for Trainium2, 8 NeuronCores.

Problem (hardcoded): input [4,2048,4096] f32, qweight int32 [512,4096] (8 int4
along K per int32), scales [32,4096], qzeros int32 [32,512] (8 int4 along N),
g_idx = arange(4096)//128 (contiguous groups), bias [4096], residual
[4,2048,4096].  out = x @ dequant(W) + bias + residual.

Sharding: data-parallel over tokens (M = B*S = 8192 rows -> 1024 rows/core);
every core keeps the full weight.

v2 strategy: the device does ONLY roofline work.  All reshuffling moves to
host numpy prep:
  - x is cast to f16 and pre-transposed/permuted on host into
    xt[kp, mi, t, j, m] = x[mi*128+m, 1024t+8kp+j], so PE lhsT tiles load
    straight from DMA (no on-device transposes, casts, or copies).
  - The GPTQ zero-point correction and bias fold into the residual on host:
    resid' = residual + bias - xg @ ((qz+1)*scales), xg = per-group sums of x.
  - Scales are pre-broadcast on host to scb[t, p, n] = scales[8t+p//16, n]
    (f16), removing the on-device E16 indicator matmuls.
Device steady state per 512-col chunk: 4 packed-weight DMAs, 32 DVE
shift+and passes, 32 DVE scale-mult passes -> wdq f16 tiles; then 8 PSUM
groups (one per 128-row M-tile) accumulate 32 matmuls each, interleaved
mi-innermost so a single dequanted weight tile feeds 8 back-to-back MMs;
epilogue: ACT copies PSUM->SBUF (frees the bank fast), DVE adds residual
in place, DMA out.
"""

import numpy as np

import concourse.bass as bass
import concourse.mybir as mybir
import concourse.tile as tile
from concourse import bacc
from concourse.alu_op_type import AluOpType
from concourse.bass_utils import run_bass_kernel_spmd

F32 = mybir.dt.float32
F16 = mybir.dt.float16
I32 = mybir.dt.int32

B, S, K, N = 4, 2048, 4096, 4096
PACK = 8
GROUP = 128
G = K // GROUP          # 32 groups
NCORES = 8
M = (B * S) // NCORES   # 1024 rows per core
CHUNK = 512
NCH = N // CHUNK        # 8
TS = K // 1024          # 4 super-tiles of 1024 k
MT = M // 128           # 8 M-tiles per core


def _build():
    nc = bacc.Bacc(name="gptq_mm2", dynamic_dma_scratch_size=2048)
    xt_d = nc.declare_dram_parameter("xt", [128, MT, TS, PACK, 128], F16, isOutput=False)
    wq_d = nc.declare_dram_parameter("wq", [K // PACK, N], I32, isOutput=False)
    scb_d = nc.declare_dram_parameter("scb", [TS, 128, N], F16, isOutput=False)
    res_d = nc.declare_dram_parameter("resid", [M, N], F32, isOutput=False)
    out_d = nc.declare_dram_parameter("out", [M, N], F32, isOutput=True)

    with tile.TileContext(nc) as tc:
        with (
            tc.tile_pool(name="const", bufs=1) as const,
            tc.tile_pool(name="pk", bufs=6) as pkp,
            tc.tile_pool(name="u", bufs=4) as up,
            tc.tile_pool(name="wdq", bufs=40) as wdqp,
            tc.tile_pool(name="rt", bufs=9) as rtp,
            tc.tile_pool(name="ob", bufs=4) as obp,
            tc.tile_pool(name="ps", bufs=8, space="PSUM") as psp,
        ):
            scb_sb = const.tile([128, TS, N], F16, tag="scb")
            xt_sb = const.tile([128, MT, TS, PACK, 128], F16, tag="xt")
            # Pre-warm the PE clock (HAM un-throttles after ~3.4us of
            # activity) with junk matmuls while the first DMAs land.
            junk = const.tile([128, CHUNK], F16, tag="junk")
            nc.vector.memset(junk[:], 0.0)
            jps = psp.tile([128, CHUNK], F32, tag="ps")
            for _ in range(40):
                nc.tensor.matmul(
                    jps[:], lhsT=junk[:, 0:128], rhs=junk[:],
                    start=True, stop=True,
                )
            # x streams on the ACT HWDGE queue so it never delays the
            # chunk-0 weight/scale DMAs on the sync queue.
            for mi in range(MT):
                nc.scalar.dma_start(out=xt_sb[:, mi], in_=xt_d[:, mi])

            def epilogue(c, mi, ps, rt):
                cs = slice(c * CHUNK, (c + 1) * CHUNK)
                ob = obp.tile([128, CHUNK], F32, tag="ob")
                nc.scalar.copy(ob[:], ps[:])
                nc.vector.tensor_tensor(
                    out=ob[:], in0=ob[:], in1=rt[:], op=AluOpType.add,
                )
                nc.sync.dma_start(
                    out=out_d[mi * 128:(mi + 1) * 128, cs], in_=ob[:]
                )

            for c in range(NCH):
                cs = slice(c * CHUNK, (c + 1) * CHUNK)
                pks = []
                for t in range(TS):
                    pk = pkp.tile([128, CHUNK], I32, tag="pk")
                    nc.sync.dma_start(out=pk[:], in_=wq_d[128 * t:128 * (t + 1), cs])
                    pks.append(pk)
                    nc.sync.dma_start(
                        out=scb_sb[:, t, cs], in_=scb_d[t][:, cs]
                    )
                rts = []
                for mi in range(MT):
                    rt = rtp.tile([128, CHUNK], F32, tag="rt")
                    nc.sync.dma_start(
                        out=rt[:], in_=res_d[mi * 128:(mi + 1) * 128, cs]
                    )
                    rts.append(rt)
                wdqs = []
                for t in range(TS):
                    for j in range(PACK):
                        u = up.tile([128, CHUNK], I32, tag="u")
                        nc.vector.tensor_scalar(
                            out=u[:], in0=pks[t][:],
                            scalar1=4 * j, scalar2=0xF,
                            op0=AluOpType.logical_shift_right,
                            op1=AluOpType.bitwise_and,
                        )
                        wdq = wdqp.tile([128, CHUNK], F16, tag="wdq")
                        nc.vector.tensor_tensor(
                            out=wdq[:], in0=u[:], in1=scb_sb[:, t, cs],
                            op=AluOpType.mult,
                        )
                        wdqs.append(wdq)
                pss = []
                for mi in range(MT):
                    ps = psp.tile([128, CHUNK], F32, tag="ps")
                    pss.append(ps)
                if c == 0:
                    # chunk 0: emit MMs in estimated-operand-arrival order
                    # (xt lands per-mi every ~2.9us on the ACT queue; wdq
                    # tiles dequant every ~0.85us) to avoid head-of-line
                    # blocking in the PE FIFO during the ramp.
                    order = sorted(
                        ((idx, mi) for idx in range(TS * PACK)
                         for mi in range(MT)),
                        key=lambda p: (max(10.0 + 2.9 * p[1],
                                           9.8 + 0.85 * p[0]), p[0], p[1]),
                    )
                    seen = [0] * MT
                    for idx, mi in order:
                        t, j = divmod(idx, PACK)
                        seen[mi] += 1
                        nc.tensor.matmul(
                            pss[mi][:],
                            lhsT=xt_sb[:, mi, t, j, :],
                            rhs=wdqs[idx][:],
                            start=(seen[mi] == 1), stop=(seen[mi] == TS * PACK),
                        )
                    for mi in range(MT):
                        epilogue(c, mi, pss[mi], rts[mi])
                elif c < NCH - 1:
                    # mi-innermost: each dequanted weight tile feeds 8
                    # back-to-back MMs.
                    for idx in range(TS * PACK):
                        t, j = divmod(idx, PACK)
                        for mi in range(MT):
                            nc.tensor.matmul(
                                pss[mi][:],
                                lhsT=xt_sb[:, mi, t, j, :],
                                rhs=wdqs[idx][:],
                                start=(idx == 0), stop=(idx == TS * PACK - 1),
                            )
                    for mi in range(MT):
                        epilogue(c, mi, pss[mi], rts[mi])
                else:
                    # last chunk: mi-outer so PSUM groups finish staggered
                    # and only one epilogue lands after the final MM.
                    for mi in range(MT):
                        for idx in range(TS * PACK):
                            t, j = divmod(idx, PACK)
                            nc.tensor.matmul(
                                pss[mi][:],
                                lhsT=xt_sb[:, mi, t, j, :],
                                rhs=wdqs[idx][:],
                                start=(idx == 0), stop=(idx == TS * PACK - 1),
                            )
                        epilogue(c, mi, pss[mi], rts[mi])

    nc.finalize()
    return nc


_NC_CACHE = None


def _get_nc():
    global _NC_CACHE
    if _NC_CACHE is None:
        _NC_CACHE = _build()
    return _NC_CACHE


def _host_prep(input, weight, weight_scales, weight_zeros, bias, residual):
    """All host-side reshuffling; returns per-core in_maps."""
    x = np.ascontiguousarray(input.reshape(B * S, K))
    x16 = x.astype(np.float16)

    # zero-point + bias correction folded into residual:
    #   out = x @ (qw*s) - xg @ ((qz+1)*s) + bias + residual
    jj = (np.arange(PACK, dtype=np.int32) * 4)
    qz = ((weight_zeros[:, :, None] >> jj[None, None, :]) & 0xF).reshape(G, N)
    nzs = (qz + 1).astype(np.float32) * weight_scales          # [G, N]
    xg = x.reshape(B * S, G, GROUP).sum(axis=2, dtype=np.float32)  # [M_all, G]
    resid2 = (
        residual.reshape(B * S, N) + bias[None, :] - xg @ nzs
    ).astype(np.float32)

    # scales broadcast: scb[t, p, n] = scales[8t + p//16, n], f16
    sc16 = weight_scales.astype(np.float16)
    scb = np.ascontiguousarray(
        np.repeat(sc16.reshape(TS, PACK, 1, N), 16, axis=2).reshape(TS, 128, N)
    )

    in_maps = []
    for ci in range(NCORES):
        rs = slice(ci * M, (ci + 1) * M)
        xc = x16[rs].reshape(MT, 128, TS, 128, PACK)   # [mi, m, t, kp, j]
        xt = np.ascontiguousarray(xc.transpose(3, 0, 2, 4, 1))  # [kp,mi,t,j,m]
        in_maps.append(dict(
            xt=xt,
            wq=np.ascontiguousarray(weight),
            scb=scb,
            resid=np.ascontiguousarray(resid2[rs]),
        ))
    return in_maps


def kernel(input, weight, weight_scales, weight_zeros, g_idx, bias, residual):
    input = np.asarray(input, dtype=np.float32)
    weight = np.ascontiguousarray(np.asarray(weight, dtype=np.int32))
    weight_scales = np.ascontiguousarray(np.asarray(weight_scales, dtype=np.float32))
    weight_zeros = np.asarray(weight_zeros, dtype=np.int32)
    g_idx = np.asarray(g_idx, dtype=np.int32)
    bias = np.asarray(bias, dtype=np.float32)
    residual = np.asarray(residual, dtype=np.float32)

    assert input.shape == (B, S, K) and weight.shape == (K // PACK, N)
    assert np.array_equal(g_idx, np.arange(K, dtype=np.int32) // GROUP), \
        "kernel assumes contiguous GPTQ groups (g_idx == arange(K)//group_size)"

    in_maps = _host_prep(input, weight, weight_scales, weight_zeros, bias, residual)
    nc = _get_nc()
    res = run_bass_kernel_spmd(nc, in_maps, core_ids=list(range(NCORES)))
    out = np.concatenate([r["out"] for r in res.results], axis=0)
    return out.reshape(B, S, N)


# revision 9
# speedup vs baseline: 1.0214x; 1.0017x over previous
"""GPTQ int4 dequant + matmul + bias + residual for Trainium2, 8 NeuronCores.

Problem (hardcoded): input [4,2048,4096] f32, qweight int32 [512,4096] (8 int4
along K per int32), scales [32,4096], qzeros int32 [32,512] (8 int4 along N),
g_idx = arange(4096)//128 (contiguous groups), bias [4096], residual
[4,2048,4096].  out = x @ dequant(W) + bias + residual.

Sharding: data-parallel over tokens (M = B*S = 8192 rows -> 1024 rows/core);
every core keeps the full weight.

v2 strategy: the device does ONLY roofline work.  All reshuffling moves to
host numpy prep:
  - x is cast to f16 and pre-transposed/permuted on host into
    xt[kp, mi, t, j, m] = x[mi*128+m, 1024t+8kp+j], so PE lhsT tiles load
    straight from DMA (no on-device transposes, casts, or copies).
  - The GPTQ zero-point correction and bias fold into the residual on host:
    resid' = residual + bias - xg @ ((qz+1)*scales), xg = per-group sums of x.
  - Scales are pre-broadcast on host to scb[t, p, n] = scales[8t+p//16, n]
    (f16), removing the on-device E16 indicator matmuls.
Device steady state per 512-col chunk: 4 packed-weight DMAs, 32 DVE
shift+and passes, 32 DVE scale-mult passes -> wdq f16 tiles; then 8 PSUM
groups (one per 128-row M-tile) accumulate 32 matmuls each, interleaved
mi-innermost so a single dequanted weight tile feeds 8 back-to-back MMs;
epilogue: ACT copies PSUM->SBUF (frees the bank fast), DVE adds residual
in place, DMA out.
"""

import numpy as np

import concourse.bass as bass
import concourse.mybir as mybir
import concourse.tile as tile
from concourse import bacc
from concourse.alu_op_type import AluOpType
from concourse.bass_utils import run_bass_kernel_spmd

F32 = mybir.dt.float32
F16 = mybir.dt.float16
I32 = mybir.dt.int32

B, S, K, N = 4, 2048, 4096, 4096
PACK = 8
GROUP = 128
G = K // GROUP          # 32 groups
NCORES = 8
M = (B * S) // NCORES   # 1024 rows per core
CHUNK = 512
NCH = N // CHUNK        # 8
TS = K // 1024          # 4 super-tiles of 1024 k
MT = M // 128           # 8 M-tiles per core


def _build():
    nc = bacc.Bacc(name="gptq_mm2", dynamic_dma_scratch_size=2048)
    xt_d = nc.declare_dram_parameter("xt", [128, MT, TS, PACK, 128], F16, isOutput=False)
    wq_d = nc.declare_dram_parameter("wq", [K // PACK, N], I32, isOutput=False)
    scb_d = nc.declare_dram_parameter("scb", [TS, 128, N], F16, isOutput=False)
    res_d = nc.declare_dram_parameter("resid", [M, N], F32, isOutput=False)
    out_d = nc.declare_dram_parameter("out", [M, N], F32, isOutput=True)

    with tile.TileContext(nc) as tc:
        with (
            tc.tile_pool(name="const", bufs=1) as const,
            tc.tile_pool(name="pk", bufs=6) as pkp,
            tc.tile_pool(name="u", bufs=4) as up,
            tc.tile_pool(name="wdq", bufs=40) as wdqp,
            tc.tile_pool(name="rt", bufs=9) as rtp,
            tc.tile_pool(name="ob", bufs=4) as obp,
            tc.tile_pool(name="ps", bufs=8, space="PSUM") as psp,
        ):
            scb_sb = const.tile([128, TS, N], F16, tag="scb")
            xt_sb = const.tile([128, MT, TS, PACK, 128], F16, tag="xt")
            # Pre-warm the PE clock (HAM un-throttles after ~3.4us of
            # activity) with junk matmuls while the first DMAs land.
            junk = const.tile([128, CHUNK], F16, tag="junk")
            nc.vector.memset(junk[:], 0.0)
            jps = psp.tile([128, CHUNK], F32, tag="ps")
            for _ in range(28):
                nc.tensor.matmul(
                    jps[:], lhsT=junk[:, 0:128], rhs=junk[:],
                    start=True, stop=True,
                )
            # x streams on the ACT HWDGE queue so it never delays the
            # chunk-0 weight/scale DMAs on the sync queue.
            for mi in range(MT):
                nc.scalar.dma_start(out=xt_sb[:, mi], in_=xt_d[:, mi])

            def epilogue(c, mi, ps, rt):
                cs = slice(c * CHUNK, (c + 1) * CHUNK)
                ob = obp.tile([128, CHUNK], F32, tag="ob")
                nc.scalar.copy(ob[:], ps[:])
                nc.vector.tensor_tensor(
                    out=ob[:], in0=ob[:], in1=rt[:], op=AluOpType.add,
                )
                nc.sync.dma_start(
                    out=out_d[mi * 128:(mi + 1) * 128, cs], in_=ob[:]
                )

            for c in range(NCH):
                cs = slice(c * CHUNK, (c + 1) * CHUNK)
                pks = []
                for t in range(TS):
                    pk = pkp.tile([128, CHUNK], I32, tag="pk")
                    nc.sync.dma_start(out=pk[:], in_=wq_d[128 * t:128 * (t + 1), cs])
                    pks.append(pk)
                    nc.sync.dma_start(
                        out=scb_sb[:, t, cs], in_=scb_d[t][:, cs]
                    )
                rts = []
                for mi in range(MT):
                    rt = rtp.tile([128, CHUNK], F32, tag="rt")
                    nc.sync.dma_start(
                        out=rt[:], in_=res_d[mi * 128:(mi + 1) * 128, cs]
                    )
                    rts.append(rt)
                wdqs = []
                for t in range(TS):
                    for j in range(PACK):
                        u = up.tile([128, CHUNK], I32, tag="u")
                        nc.vector.tensor_scalar(
                            out=u[:], in0=pks[t][:],
                            scalar1=4 * j, scalar2=0xF,
                            op0=AluOpType.logical_shift_right,
                            op1=AluOpType.bitwise_and,
                        )
                        wdq = wdqp.tile([128, CHUNK], F16, tag="wdq")
                        nc.vector.tensor_tensor(
                            out=wdq[:], in0=u[:], in1=scb_sb[:, t, cs],
                            op=AluOpType.mult,
                        )
                        wdqs.append(wdq)
                pss = []
                for mi in range(MT):
                    ps = psp.tile([128, CHUNK], F32, tag="ps")
                    pss.append(ps)
                if c == 0:
                    # chunk 0: emit MMs in estimated-operand-arrival order
                    # (xt lands per-mi every ~2.9us on the ACT queue; wdq
                    # tiles dequant every ~0.85us) to avoid head-of-line
                    # blocking in the PE FIFO during the ramp.
                    order = sorted(
                        ((idx, mi) for idx in range(TS * PACK)
                         for mi in range(MT)),
                        key=lambda p: (max(10.0 + 3.4 * p[1],
                                           9.8 + 0.85 * p[0]), p[0], p[1]),
                    )
                    seen = [0] * MT
                    for idx, mi in order:
                        t, j = divmod(idx, PACK)
                        seen[mi] += 1
                        nc.tensor.matmul(
                            pss[mi][:],
                            lhsT=xt_sb[:, mi, t, j, :],
                            rhs=wdqs[idx][:],
                            start=(seen[mi] == 1), stop=(seen[mi] == TS * PACK),
                        )
                    for mi in range(MT):
                        epilogue(c, mi, pss[mi], rts[mi])
                elif c < NCH - 1:
                    # mi-innermost: each dequanted weight tile feeds 8
                    # back-to-back MMs.
                    for idx in range(TS * PACK):
                        t, j = divmod(idx, PACK)
                        for mi in range(MT):
                            nc.tensor.matmul(
                                pss[mi][:],
                                lhsT=xt_sb[:, mi, t, j, :],
                                rhs=wdqs[idx][:],
                                start=(idx == 0), stop=(idx == TS * PACK - 1),
                            )
                    for mi in range(MT):
                        epilogue(c, mi, pss[mi], rts[mi])
                else:
                    # last chunk: mi-outer so PSUM groups finish staggered;
                    # epilogues split in 256-col halves to pipeline the
                    # final PSUM drain with the out DMA.
                    for mi in range(MT):
                        for idx in range(TS * PACK):
                            t, j = divmod(idx, PACK)
                            nc.tensor.matmul(
                                pss[mi][:],
                                lhsT=xt_sb[:, mi, t, j, :],
                                rhs=wdqs[idx][:],
                                start=(idx == 0), stop=(idx == TS * PACK - 1),
                            )
                        for h in range(2):
                            hs = slice(h * 256, (h + 1) * 256)
                            hcs = slice(c * CHUNK + h * 256,
                                        c * CHUNK + (h + 1) * 256)
                            obh = obp.tile([128, 256], F32, tag="obh")
                            nc.scalar.copy(obh[:], pss[mi][:, hs])
                            nc.vector.tensor_tensor(
                                out=obh[:], in0=obh[:], in1=rts[mi][:, hs],
                                op=AluOpType.add,
                            )
                            nc.sync.dma_start(
                                out=out_d[mi * 128:(mi + 1) * 128, hcs],
                                in_=obh[:],
                            )

    nc.finalize()
    return nc


_NC_CACHE = None


def _get_nc():
    global _NC_CACHE
    if _NC_CACHE is None:
        _NC_CACHE = _build()
    return _NC_CACHE


def _host_prep(input, weight, weight_scales, weight_zeros, bias, residual):
    """All host-side reshuffling; returns per-core in_maps."""
    x = np.ascontiguousarray(input.reshape(B * S, K))
    x16 = x.astype(np.float16)

    # zero-point + bias correction folded into residual:
    #   out = x @ (qw*s) - xg @ ((qz+1)*s) + bias + residual
    jj = (np.arange(PACK, dtype=np.int32) * 4)
    qz = ((weight_zeros[:, :, None] >> jj[None, None, :]) & 0xF).reshape(G, N)
    nzs = (qz + 1).astype(np.float32) * weight_scales          # [G, N]
    xg = x.reshape(B * S, G, GROUP).sum(axis=2, dtype=np.float32)  # [M_all, G]
    resid2 = (
        residual.reshape(B * S, N) + bias[None, :] - xg @ nzs
    ).astype(np.float32)

    # scales broadcast: scb[t, p, n] = scales[8t + p//16, n], f16
    sc16 = weight_scales.astype(np.float16)
    scb = np.ascontiguousarray(
        np.repeat(sc16.reshape(TS, PACK, 1, N), 16, axis=2).reshape(TS, 128, N)
    )

    in_maps = []
    for ci in range(NCORES):
        rs = slice(ci * M, (ci + 1) * M)
        xc = x16[rs].reshape(MT, 128, TS, 128, PACK)   # [mi, m, t, kp, j]
        xt = np.ascontiguousarray(xc.transpose(3, 0, 2, 4, 1))  # [kp,mi,t,j,m]
        in_maps.append(dict(
            xt=xt,
            wq=np.ascontiguousarray(weight),
            scb=scb,
            resid=np.ascontiguousarray(resid2[rs]),
        ))
    return in_maps


def kernel(input, weight, weight_scales, weight_zeros, g_idx, bias, residual):
    input = np.asarray(input, dtype=np.float32)
    weight = np.ascontiguousarray(np.asarray(weight, dtype=np.int32))
    weight_scales = np.ascontiguousarray(np.asarray(weight_scales, dtype=np.float32))
    weight_zeros = np.asarray(weight_zeros, dtype=np.int32)
    g_idx = np.asarray(g_idx, dtype=np.int32)
    bias = np.asarray(bias, dtype=np.float32)
    residual = np.asarray(residual, dtype=np.float32)

    assert input.shape == (B, S, K) and weight.shape == (K // PACK, N)
    assert np.array_equal(g_idx, np.arange(K, dtype=np.int32) // GROUP), \
        "kernel assumes contiguous GPTQ groups (g_idx == arange(K)//group_size)"

    in_maps = _host_prep(input, weight, weight_scales, weight_zeros, bias, residual)
    nc = _get_nc()
    res = run_bass_kernel_spmd(nc, in_maps, core_ids=list(range(NCORES)))
    out = np.concatenate([r["out"] for r in res.results], axis=0)
    return out.reshape(B, S, N)
